# revision 51
# baseline (speedup 1.0000x reference)
"""Trainium2 Bass kernel for nn_AttentionHead (single-head attention with
pre-softmax tril zeroing). B=8, S=2048, E=1024, H=64.

Sharding: data-parallel over batch - one batch element per NeuronCore,
no collectives.

Host marshaling: inputs cast to bf16 and transposed to [E, S], laid out
per-partition-contiguous so every DMA is a single descriptor per
partition. DMA order interleaves y (q-chunks of 512) and x (kv s-block
pairs of 512 = 4 key blocks) so projections and attention waves start
while later blocks are still in flight.

Math per core (biases are zero in this problem; a slow path DMAs them
if any are nonzero):
  q = y@Wq ; k' = x@(Wk/8) ; v = x@Wv
  ST[sk, sq] = k'_blk . q   (transposed scores), lower blocks only
  diag-block masked entries fixed to exp(0)=1 post-exp (Pool
  affine_select fill=1.0), matching the reference tril-then-softmax
  never-materialized upper blocks are closed form: num += suffix_sum(v),
  Z += count (v sums fall out of the kv-evac accum_out for free)
  softmax denominator via an augmented ones-column in v (pv row 64)
  out = attn @ v, normalized after a PE transpose back to [s, h]
  (one strided reciprocal + one broadcast multiply per 512-row chunk)

Engine placement: PE does matmuls/transposes only; Act does exp + the
finish bias-add; DVE does evacs/normalize; Pool does the diag mask fix.
Partition layout trick: v lives on partitions 0:64 (kvT rows 0:64), k
on 64:128, and q is projected straight into PSUM partitions 64:128
(tile_position col 64), so score matmuls run with both operands at base
partition 64 and nothing ever needs a cross-partition copy.

Attention is software-pipelined: each item emits score matmul + exp
first and parks its PV matmul, which is flushed at the next item - so
the PE never sits behind the Activation engine.
"""

import numpy as np

import concourse.bass as bass
import concourse.mybir as mybir
from concourse.tile import TileContext

S, E, H = 2048, 1024, 64
ECH = E // 128  # 8 e-chunks
NQ = 4          # q-chunks of 512
NKB = S // 128  # 16 key blocks
NXP = 4         # x s-block pairs of 512 cols
WPE = 128 + H   # packed weight cols per echunk: [Wv|Wk'] then Wq
F32 = mybir.dt.float32
BF16 = mybir.dt.bfloat16
AF = mybir.ActivationFunctionType
ALU = mybir.AluOpType

_SPLIT_COUNTER = [0]


def _split_multi_waits(nc, ev_cap=1):
    """This container's walrus build accepts at most 1 sem-wait per
    instruction (2 on EventSemaphore); move excess waits onto EvSem
    instructions inserted just before, on the same engine."""
    for f in nc.m.functions:
        for bb in f.blocks:
            ins_list = bb.instructions
            need = False
            for ins in ins_list:
                si = ins.sync_info
                if si is None:
                    continue
                cap = 2 if isinstance(ins, mybir.InstEventSemaphore) else 1
                if len(si.on_wait) > cap:
                    need = True
                    break
            if not need:
                continue
            new_list = []
            for ins in ins_list:
                si = ins.sync_info
                cap = 2 if isinstance(ins, mybir.InstEventSemaphore) else 1
                if si is not None and len(si.on_wait) > cap:
                    waits = list(si.on_wait)
                    keep = waits[-cap:]
                    head = waits[:-cap]
                    for i in range(0, len(head), ev_cap):
                        _SPLIT_COUNTER[0] += 1
                        ev = mybir.InstEventSemaphore(
                            name=f"EVSPLIT-{_SPLIT_COUNTER[0]}",
                            engine=ins.engine,
                            ins=[],
                            outs=[],
                            sync_info=mybir.SyncInfo(
                                on_wait=head[i:i + ev_cap], on_update=[]
                            ),
                        )
                        nc.register_instruction(ev)
                        new_list.append(ev)
                    ins.sync_info = mybir.SyncInfo(
                        on_wait=keep, on_update=list(si.on_update)
                    )
                new_list.append(ins)
            bb.instructions = new_list


def _build(use_bias):
    nc = bass.Bass()
    x_ext = nc.declare_dram_parameter("x", [128, NXP * ECH * 512], BF16,
                                      isOutput=False)
    y_ext = nc.declare_dram_parameter("y", [128, NQ * ECH * 512], BF16,
                                      isOutput=False)
    # per echunk: [Wv | Wk/8 | Wq] = 192 cols
    w_ext = nc.declare_dram_parameter("w", [128, ECH * WPE], BF16,
                                      isOutput=False)
    if use_bias:
        bvk_ext = nc.declare_dram_parameter("bvk", [128, 1], F32,
                                            isOutput=False)
        bq_ext = nc.declare_dram_parameter("bq", [H, 1], F32, isOutput=False)
    out_ext = nc.declare_dram_parameter("out", [S, H], F32, isOutput=True)

    with TileContext(nc) as tc:
        with (
            tc.tile_pool(name="consts", bufs=1) as consts,
            tc.tile_pool(name="bigin", bufs=1) as bigin,
            tc.tile_pool(name="qkv", bufs=1) as qkvp,
            tc.tile_pool(name="expp", bufs=4) as expp,
            tc.tile_pool(name="outp", bufs=2) as outp,
            tc.tile_pool(name="ps", bufs=1, space="PSUM") as ps,
        ):
            # ---- DMA issues first on SP so rings start streaming ----
            wsb = consts.tile([128, ECH * WPE], BF16, tag="wsb")
            nc.scalar.dma_start(out=wsb, in_=w_ext[:, :])
            if use_bias:
                bvk_sb = consts.tile([128, 1], F32, tag="bvk")
                nc.sync.dma_start(out=bvk_sb, in_=bvk_ext[:, :])
                bq_sb = consts.tile([H, 1], F32, tag="bq")
                nc.sync.dma_start(out=bq_sb, in_=bq_ext[:, :])

            x_sb = bigin.tile([128, NXP * ECH * 512], BF16, tag="x_sb")
            y_sb = bigin.tile([128, NQ * ECH * 512], BF16, tag="y_sb")

            def dma_x2(p):  # s-block pair p: key blocks 4p..4p+3 (1MB)
                nc.sync.dma_start(
                    out=x_sb[:, p * 4096:(p + 1) * 4096],
                    in_=x_ext[:, p * 4096:(p + 1) * 4096],
                )

            def dma_y(c):
                nc.sync.dma_start(
                    out=y_sb[:, c * 4096:(c + 1) * 4096],
                    in_=y_ext[:, c * 4096:(c + 1) * 4096],
                )

            nc.sync.dma_start(out=y_sb[:, 0:1024], in_=y_ext[:, 0:1024])
            nc.sync.dma_start(out=y_sb[:, 1024:4096], in_=y_ext[:, 1024:4096])
            dma_x2(0)
            dma_y(1); dma_x2(1)
            dma_y(2); dma_y(3)
            dma_x2(2); dma_x2(3)

            # ---- constants ----
            wdum = consts.tile([128, 128], BF16, tag="wdum")
            nc.vector.memset(wdum, 0.0)
            ident_bf = consts.tile([128, 128], BF16, tag="ident_bf")
            nc.vector.memset(ident_bf, 1.0)
            nc.gpsimd.affine_select(
                out=ident_bf, in_=ident_bf,
                pattern=[[-1, 128]], channel_multiplier=1, base=0,
                compare_op=ALU.is_equal, fill=0.0,
            )

            # ---- persistent data tiles ----
            # kvT: rows 0:64 = vT, rows 64:128 = kT (scaled)
            kvT = qkvp.tile([128, S], BF16, tag="kvT")
            # qT2: rows 64:128 = qT (rows 0:64 unused)
            qT2 = qkvp.tile([128, S], BF16, tag="qT2")
            # v natural + ones column per key block
            v_aug = qkvp.tile([128, NKB * (H + 1)], BF16, tag="v_aug")
            nc.vector.memset(v_aug, 1.0)
            # per-xpair v column sums (rows 0:64 valid)
            bsums = qkvp.tile([128, NXP], F32, tag="bsums")
            # suffix sums + count row (partitions 0:65 used)
            vsufs = qkvp.tile([H + 1, NQ], F32, tag="vsufs")

            # pv accumulators for chunk pairs (0,1) and (2,3): [65, 1024]
            pv01 = ps.tile([H + 1, 1024], F32, tag="pv01", name="pv01")
            pv23 = ps.tile([H + 1, 1024], F32, tag="pv23", name="pv23")

            def pv_ap(c, lo, hi):  # cols [lo,hi) of chunk c's 512-range
                t = pv01 if c < 2 else pv23
                off = (c % 2) * 512
                return t[:, off + lo:off + hi]

            def qproj(c):
                acc = ps.tile([128, 512], F32, tag="pa", bufs=1,
                              name=f"qacc_{c}")
                for e in range(ECH):
                    nc.tensor.matmul(
                        acc[64:128, :],
                        lhsT=wsb[:, e * WPE + 128:(e + 1) * WPE],
                        rhs=y_sb[:, (c * ECH + e) * 512:(c * ECH + e + 1) * 512],
                        start=(e == 0),
                        stop=(e == ECH - 1),
                    )
                nc.vector.tensor_scalar_add(
                    out=qT2[64:128, c * 512:(c + 1) * 512],
                    in0=acc[64:128, :],
                    scalar1=bq_sb if use_bias else 0.0,
                )

            def kvproj(p):  # x s-block pair p -> kvT cols, v_aug blocks
                acc = ps.tile([128, 512], F32, tag="pa", bufs=1,
                              name=f"kvacc_{p}")
                for e in range(ECH):
                    nc.tensor.matmul(
                        acc,
                        lhsT=wsb[:, e * WPE:e * WPE + 128],
                        rhs=x_sb[:, (p * ECH + e) * 512:(p * ECH + e + 1) * 512],
                        start=(e == 0),
                        stop=(e == ECH - 1),
                    )
                nc.vector.tensor_scalar(
                    out=kvT[:, p * 512:(p + 1) * 512],
                    in0=acc,
                    scalar1=bvk_sb if use_bias else 0.0,
                    scalar2=0.0,
                    op0=ALU.add, op1=ALU.add,
                    accum_out=bsums[:, p:p + 1],
                )
                # natural-layout v via DMA XBAR transpose (off the PE)
                for b in range(4 * p, 4 * p + 4):
                    nc.sync.dma_start_transpose(
                        out=v_aug[:, b * (H + 1):b * (H + 1) + H],
                        in_=kvT[0:64, b * 128:(b + 1) * 128],
                    )

            # software-pipelined attention: scores/exp of item n emit
            # before PV of item n-1 so the PE never waits on Act
            pend = [None]

            def flush_pv():
                if pend[0] is not None:
                    for mm in pend[0]:
                        nc.tensor.matmul(**mm)
                    pend[0] = None

            def attn1(b, c, diag, stop):
                d = (b - 4 * c) * 128 if diag else 0
                st = ps.tile([128, 512], F32, tag="st", bufs=3,
                             name=f"st_{b}_{c}")
                nc.tensor.matmul(
                    st[:, d:512],
                    lhsT=kvT[64:128, b * 128:(b + 1) * 128],
                    rhs=qT2[64:128, c * 512 + d:(c + 1) * 512],
                    start=True, stop=True,
                )
                ex = expp.tile([128, 512], BF16, tag="ex", bufs=8,
                               name=f"ex_{b}_{c}")
                nc.scalar.activation(out=ex[:, d:512], in_=st[:, d:512],
                                     func=AF.Exp)
                if diag:
                    # masked entries (key > query) -> exp(0)=1
                    nc.gpsimd.affine_select(
                        out=ex[:, 0:d + 128], in_=ex[:, 0:d + 128],
                        pattern=[[1, d + 128]], channel_multiplier=-1,
                        base=-d, compare_op=ALU.is_ge, fill=1.0,
                    )
                flush_pv()
                va = v_aug[:, b * (H + 1):(b + 1) * (H + 1)]
                pend[0] = [dict(
                    out=pv_ap(c, 0, 512), lhsT=va, rhs=ex,
                    start=(b == 0), stop=stop,
                )]

            def attn(b, cl, ch, diag):
                attn1(b, cl, diag, stop=(diag and b == 4 * cl + 3))
                if ch > cl:
                    attn1(b, ch, False, stop=False)

            # ---- emission schedule ----
            # warm-up burst: keep the PE busy while the first inputs
            # stream in, so the activity monitor ramps before real work
            # (wdum is memset-only - no Pool dependency - so this starts
            # as soon as the DVE preamble finishes)
            _fill_n = [0]

            def filler(n):
                # dependency-free matmuls that absorb DMA-paced PE idle
                # so the activity monitor never re-throttles the core
                _fill_n[0] += 1
                wt = ps.tile([128, 512], F32, tag="pa", bufs=1,
                             name=f"warm_{_fill_n[0]}")
                for _ in range(n):
                    nc.tensor.matmul(wt[:, 0:128], lhsT=wdum,
                                     rhs=wdum, start=True, stop=True)

            warm = ps.tile([128, 512], F32, tag="st", bufs=3, name="warm")
            for _ in range(44):
                nc.tensor.matmul(warm[:, 0:128], lhsT=wdum,
                                 rhs=wdum, start=True, stop=True)
            qproj(0)
            filler(6)
            kvproj(0)   # key blocks 0..3
            filler(9)
            qproj(1)
            for b in range(4):
                attn(b, 0, 1, diag=True)
            filler(7)
            kvproj(1)   # key blocks 4..7 (x pair 1 lands before y2/y3)
            for b in range(4, 8):
                attn(b, 1, 1, diag=True)
            qproj(2)
            qproj(3)
            for b in range(4):
                attn(b, 2, 3, diag=False)
            for b in range(4, 8):
                attn(b, 2, 3, diag=False)
            kvproj(2)   # key blocks 8..11
            for b in range(8, 12):
                attn(b, 2, 3, diag=True)
            kvproj(3)   # key blocks 12..15
            for b in range(12, 16):
                attn(b, 3, 3, diag=True)
            flush_pv()

            # ---- tail: suffix sums, normalize, store ----
            for c in range(NQ - 1):
                nc.vector.reduce_sum(
                    out=vsufs[0:H, c:c + 1],
                    in_=bsums[0:H, c + 1:NXP],
                    axis=mybir.AxisListType.X,
                )
                nc.vector.memset(vsufs[H:H + 1, c:c + 1],
                                 float((NQ - 1 - c) * 512))
            nc.vector.memset(vsufs[0:H + 1, NQ - 1:NQ], 0.0)

            # bias-adds first (Act), then PE transposes (bf16), then
            # normalize (DVE/Act split), then store
            PW = H + 2  # 2-byte elems: keep per-j offsets 4B aligned
            sbns = []
            for c in range(NQ):
                sbn = outp.tile([H + 1, 512], BF16, tag="sbn", bufs=4,
                                name=f"sbn_{c}")
                nc.scalar.activation(
                    out=sbn, in_=pv_ap(c, 0, 512),
                    func=AF.Identity,
                    bias=vsufs[0:H + 1, c:c + 1],
                )
                sbns.append(sbn)
            pts = []
            for c in range(NQ):
                pt = ps.tile([128, 4 * PW], BF16, tag="st", bufs=3,
                             name=f"pt_{c}")
                for j in range(4):
                    nc.tensor.transpose(
                        pt[:, j * PW:j * PW + H + 1],
                        sbns[c][:, j * 128:(j + 1) * 128],
                        ident_bf[0:H + 1, 0:H + 1],
                    )
                pts.append(pt)
            for c in range(NQ):
                of = outp.tile([128, 4 * H], F32, tag="of", bufs=2,
                               name=f"of_{c}")
                rcpv = outp.tile([128, 4], F32, tag="rcpv", bufs=4,
                                 name=f"rcpv_{c}")
                pt3 = pts[c].rearrange("p (j w) -> p j w", w=PW)
                nc.vector.reciprocal(rcpv[:, :, None], pt3[:, :, H:H + 1])
                nc.vector.tensor_mul(
                    out=of.rearrange("p (j h) -> p j h", h=H),
                    in0=pt3[:, :, 0:H],
                    in1=rcpv[:, :, None].broadcast_to([128, 4, H]),
                )
                nc.sync.dma_start(
                    out=out_ext[c * 512:(c + 1) * 512, :].rearrange(
                        "(j p) h -> p j h", p=128),
                    in_=of.rearrange("p (j h) -> p j h", h=H),
                )

    _split_multi_waits(nc)
    return nc


LAST_EXEC_TIME_NS = None
_CACHE = {}


def kernel(x, y, Wq, bq, Wk, bk, Wv, bv):
    """Full-input entry point: shards batch over 8 NeuronCores (one batch
    element per core), runs the Bass kernel, gathers the full output."""
    global LAST_EXEC_TIME_NS
    import os

    import ml_dtypes
    from concourse.bass_utils import run_bass_kernel_spmd

    bf16 = ml_dtypes.bfloat16
    x = np.asarray(x, np.float32)
    y = np.asarray(y, np.float32)
    bq_f = np.asarray(bq, np.float32).reshape(-1)
    bk_f = np.asarray(bk, np.float32).reshape(-1)
    bv_f = np.asarray(bv, np.float32).reshape(-1)
    use_bias = bool(np.any(bq_f) or np.any(bk_f) or np.any(bv_f))

    key = ("nc", use_bias)
    if key not in _CACHE:
        _CACHE[key] = _build(use_bias)
    nc = _CACHE[key]

    # [E,S] transposed, then per-partition contiguous: partition p holds,
    # for each 512-col block, rows e*128+p of xT
    xm = np.ascontiguousarray(
        x.transpose(0, 2, 1).reshape(8, ECH, 128, NXP, 512)
        .transpose(0, 2, 3, 1, 4).reshape(8, 128, NXP * ECH * 512)
    ).astype(bf16)
    ym = np.ascontiguousarray(
        y.transpose(0, 2, 1).reshape(8, ECH, 128, NQ, 512)
        .transpose(0, 2, 3, 1, 4).reshape(8, 128, NQ * ECH * 512)
    ).astype(bf16)
    # weights: [Wv | Wk/8 | Wq] per echunk, row p = dims e*128+p
    w_all = np.concatenate(
        [np.asarray(Wv, np.float32),
         np.asarray(Wk, np.float32) * 0.125,
         np.asarray(Wq, np.float32)],
        axis=1,
    ).reshape(ECH, 128, WPE).transpose(1, 0, 2).reshape(128, ECH * WPE)
    wm = np.ascontiguousarray(w_all).astype(bf16)

    in_maps = []
    for b in range(8):
        m = {
            "x": np.ascontiguousarray(xm[b]),
            "y": np.ascontiguousarray(ym[b]),
            "w": wm,
        }
        if use_bias:
            m["bvk"] = np.ascontiguousarray(
                np.concatenate([bv_f, bk_f * 0.125]).reshape(128, 1)
            ).astype(np.float32)
            m["bq"] = np.ascontiguousarray(
                bq_f.reshape(H, 1)).astype(np.float32)
        in_maps.append(m)

    trace = bool(os.environ.get("ATTN_TRACE"))
    res = run_bass_kernel_spmd(nc, in_maps, core_ids=list(range(8)), trace=trace)
    if trace:
        LAST_EXEC_TIME_NS = res.exec_time_ns
        reps = int(os.environ.get("ATTN_REPEAT", "0"))
        times = [res.exec_time_ns]
        for _ in range(reps):
            r2 = run_bass_kernel_spmd(nc, in_maps, core_ids=list(range(8)),
                                      trace=True)
            times.append(r2.exec_time_ns)
        if reps:
            print(f"exec times: {times}")
            LAST_EXEC_TIME_NS = min(t for t in times if t)
    return np.stack([res.results[i]["out"] for i in range(8)]).astype(np.float32)
